# revision 29
# baseline (speedup 1.0000x reference)
"""GCN 3-layer kernel for Trainium2, 8-core SPMD.

Math (per layer, PyG GCN convention with self-loops, factorized):
    deg[d]  = indegree(d) + 1;  dinv = deg^-1/2
    y       = dinv[:,None] * (h @ W)                    (per-node scale)
    agg[d]  = sum_{e: dst[e]=d} y[src[e]]  + y[d]       (self-loop as edge)
    h_next  = dinv[:,None] * agg + b                    (+ relu on last layer)

Distribution: destination-sharded across 8 cores (6272 = 49*128 node slots
per core, padded to 50176 total).  The per-layer y table is exchanged in
TWO AllGather halves split by window (windows 0-24 -> table A of 25600
rows, windows 25-48 -> table B of 24576 rows) so the first half of the
exchange overlaps the tail of the previous layer's aggregation: y for
layer L+1 is computed inside layer L's per-window epilogue (phase-A fold),
and AG_A fires as soon as window 24's y is out.

Aggregation: per 128-dst window, gather message rows with dma_gather
(4 SWDGE queues; stream A split over q0/q1, stream B over q2/q3; indices
sorted by source row for DRAM page locality) and scatter-add them with
one-hot matmuls on the PE (PSUM accumulation per window).  One-hot blocks
S[e, d, j] = (dl[e, j] == d) are built in batches on the DVE: one
tensor_tensor(is_equal) per (window, stream) with a stride-0 broadcast of
dl against a materialized iota, consumed by the PE as strided slices
S[:, :, j].

Gather indices are int16: both tables are < 32768 rows, so no base-offset
tricks are needed.
"""

import numpy as np
import ml_dtypes

N_NODES = 50000
N_CORES = 8
PER_CORE = 6272            # 49 * 128
N_PAD = PER_CORE * N_CORES # 50176
N_WIN = PER_CORE // 128    # 49
A_WINS = 32                # windows 0..31 -> table A (32*128*8 = 32768 rows)
B_WINS = N_WIN - A_WINS    # windows 32..48 -> table B
A_PC = A_WINS * 128        # 3200 rows per core in A
B_PC = B_WINS * 128        # 3072 rows per core in B
A_ROWS = A_PC * N_CORES    # 25600
B_ROWS = B_PC * N_CORES    # 24576
F = 128                    # feature width (layer3 padded 64->128)
F_OUT = 64
GROUP_WINDOWS = 4          # windows per gather chunk

BF16 = ml_dtypes.bfloat16


def _wrap_idx16(idx: np.ndarray) -> np.ndarray:
    """Wrap a flat int16 index stream into the [128, n/16] layout dma_gather
    expects (element i at [i%16, i//16], replicated across the 8 groups of
    16 partitions)."""
    n = len(idx)
    assert n % 128 == 0
    cols = n // 16
    out = np.empty((128, cols), np.int16)
    w = idx.reshape(cols, 16).T  # [16, cols]
    for g in range(8):
        out[g * 16:(g + 1) * 16, :] = w
    return out


def _preprocess(edge_index: np.ndarray):
    """Host-side graph prep: degree norm, dst-sharding, per-window edge
    streams (A/B by source window), block padding shared across cores.

    Nodes are dealt into (core, window) slots round-robin by in-degree so
    every window carries a near-equal edge load across cores -- the shared
    (max-over-core) block padding shrinks accordingly."""
    src0 = edge_index[0].astype(np.int64)
    dst0 = edge_index[1].astype(np.int64)
    deg0 = np.bincount(dst0, minlength=N_NODES)
    order_n = np.argsort(-deg0, kind="stable")
    nbuck = N_CORES * N_WIN
    i = np.arange(N_NODES)
    bw = i % nbuck
    rank = i // nbuck
    slots = (bw // N_WIN) * PER_CORE + (bw % N_WIN) * 128 + rank
    new_pos = np.empty(N_NODES, np.int64)
    new_pos[order_n] = slots
    src = new_pos[src0]
    dst = new_pos[dst0]
    deg = np.bincount(dst, minlength=N_PAD).astype(np.float64) + 1.0
    dinv_pad = (1.0 / np.sqrt(deg)).astype(np.float32)

    # self-loops are applied on-chip from the local y window (no gather)
    src_a, dst_a = src, dst

    core_of = dst_a // PER_CORE
    win_of = (dst_a % PER_CORE) // 128
    dloc_of = dst_a % 128
    src_core = src_a // PER_CORE
    src_loc = src_a % PER_CORE
    is_lo = src_loc < A_PC          # stream A: source window < 25
    # relabeled gather rows in each half-table
    gidx_a = src_core * A_PC + src_loc
    gidx_b = src_core * B_PC + (src_loc - A_PC)

    # sort once by (core, window)
    order = np.lexsort((dst_a, win_of, core_of))
    core_s, win_s, dloc_s, lo_s = (
        core_of[order], win_of[order], dloc_of[order], is_lo[order])
    ga_s, gb_s = gidx_a[order], gidx_b[order]

    # per (core, window, stream) counts
    counts = np.zeros((N_CORES, N_WIN, 2), np.int64)
    np.add.at(counts, (core_s, win_s, (~lo_s).astype(np.int64)), 1)
    # shared block counts per window (max over cores), at least 1 block
    blk_lo = np.maximum(1, -(-counts[:, :, 0].max(axis=0) // 128))  # [N_WIN]
    blk_hi = np.maximum(1, -(-counts[:, :, 1].max(axis=0) // 128))  # [N_WIN]

    # slot offsets within each stream
    off_lo = np.concatenate([[0], np.cumsum(blk_lo * 128)])
    off_hi = np.concatenate([[0], np.cumsum(blk_hi * 128)])
    n_lo, n_hi = int(off_lo[-1]), int(off_hi[-1])

    # fill per-core padded streams
    idx_lo = np.zeros((N_CORES, n_lo), np.int16)
    idx_hi = np.zeros((N_CORES, n_hi), np.int16)
    dl_lo = np.full((N_CORES, n_lo), 999.0, np.float32)
    dl_hi = np.full((N_CORES, n_hi), 999.0, np.float32)

    # boundaries of (core, window) groups in the sorted arrays
    keys = core_s * N_WIN + win_s
    bounds = np.searchsorted(keys, np.arange(N_CORES * N_WIN + 1))
    for c in range(N_CORES):
        for w in range(N_WIN):
            k = c * N_WIN + w
            sl = slice(bounds[k], bounds[k + 1])
            s_dl = dloc_s[sl]; s_lo = lo_s[sl]
            lo_src = ga_s[sl][s_lo]; lo_dl = s_dl[s_lo]
            hi_src = gb_s[sl][~s_lo]; hi_dl = s_dl[~s_lo]
            # sort each stream by source row: ascending DRAM addresses give
            # the memory controller page locality during the gather
            o_lo = np.argsort(lo_src, kind="stable")
            lo_src, lo_dl = lo_src[o_lo], lo_dl[o_lo]
            o_hi = np.argsort(hi_src, kind="stable")
            hi_src, hi_dl = hi_src[o_hi], hi_dl[o_hi]
            o = off_lo[w]
            idx_lo[c, o:o + len(lo_src)] = lo_src.astype(np.int16)
            dl_lo[c, o:o + len(lo_src)] = lo_dl
            o = off_hi[w]
            idx_hi[c, o:o + len(hi_src)] = hi_src.astype(np.int16)
            dl_hi[c, o:o + len(hi_src)] = hi_dl

    return (dinv_pad, blk_lo, blk_hi, off_lo, off_hi, idx_lo, idx_hi,
            dl_lo, dl_hi, new_pos)


def _build_and_run(inputs_np, dinv_pad, blk_lo, blk_hi, off_lo, off_hi,
                   idx_lo, idx_hi, dl_lo, dl_hi, new_pos,
                   trace=False, sim=False):
    import concourse.bacc as bacc
    import concourse.mybir as mybir
    from concourse.tile import TileContext
    from concourse import bass, bass_utils, library_config
    from concourse.masks import make_identity

    x = inputs_np["x"]
    Ws = [np.asarray(inputs_np[k], np.float32) for k in ("W1", "W2", "W3")]
    bs = [np.asarray(inputs_np[k], np.float32) for k in ("b1", "b2", "b3")]
    # pad W3/b3 to 128 output features
    W3p = np.zeros((F, F), np.float32); W3p[:, :F_OUT] = Ws[2]
    b3p = np.zeros(F, np.float32); b3p[:F_OUT] = bs[2]
    Ws[2], bs[2] = W3p, b3p

    n_lo, n_hi = idx_lo.shape[1], idx_hi.shape[1]
    # gather groups of GROUP_WINDOWS windows
    groups = [list(range(g, min(g + GROUP_WINDOWS, N_WIN)))
              for g in range(0, N_WIN, GROUP_WINDOWS)]
    glo = [(int(off_lo[g[0]]), int(off_lo[g[-1] + 1])) for g in groups]
    ghi = [(int(off_hi[g[0]]), int(off_hi[g[-1] + 1])) for g in groups]
    cap_lo = max(b - a for a, b in glo) // 128
    cap_hi = max(b - a for a, b in ghi) // 128
    NBMAX = max(cap_lo, cap_hi)   # iota depth for group-level S builds

    nc = bacc.Bacc("TRN2", target_bir_lowering=False, debug=False,
                   num_devices=N_CORES, num_swdge_queues=4,
                   dynamic_dma_scratch_size=24576)
    dt = mybir.dt

    # ---- kernel I/O -----------------------------------------------------
    t_xT = nc.dram_tensor("xT_own", [128, PER_CORE], dt.bfloat16, kind="ExternalInput")
    t_W = [nc.dram_tensor(f"W{i+1}m", [F, F], dt.float32, kind="ExternalInput") for i in range(3)]
    t_b = [nc.dram_tensor(f"b{i+1}m", [128, F], dt.float32, kind="ExternalInput") for i in range(3)]
    t_dinv = nc.dram_tensor("dinv_own", [128, N_WIN], dt.float32, kind="ExternalInput")
    t_iota3 = nc.dram_tensor("iota3", [128, 128, NBMAX], dt.bfloat16, kind="ExternalInput")
    t_ilo = nc.dram_tensor("idx_lo", [128, n_lo // 16], dt.int16, kind="ExternalInput")
    t_ihi = nc.dram_tensor("idx_hi", [128, n_hi // 16], dt.int16, kind="ExternalInput")
    t_dlo = nc.dram_tensor("dl_lo", [128, n_lo // 128], dt.bfloat16, kind="ExternalInput")
    t_dhi = nc.dram_tensor("dl_hi", [128, n_hi // 128], dt.bfloat16, kind="ExternalInput")
    t_out = nc.dram_tensor("h_out", [PER_CORE, F_OUT], dt.float32, kind="ExternalOutput")

    with TileContext(nc) as tc:
        nc.gpsimd.load_library(library_config.mlp)
        with tc.tile_pool(name="const", bufs=1) as cpool, \
             tc.tile_pool(name="state", bufs=1) as spool, \
             tc.tile_pool(name="gath", bufs=3) as gpool, \
             tc.tile_pool(name="sbld", bufs=2) as sbld, \
             tc.tile_pool(name="work", bufs=7) as wpool, \
             tc.tile_pool(name="psA", bufs=3, space="PSUM") as psA, \
             tc.tile_pool(name="psB", bufs=3, space="PSUM") as psB, \
             tc.tile_pool(name="psT", bufs=2, space="PSUM") as psT, \
             tc.tile_pool(name="dram", bufs=1, space="DRAM") as dpool:

            # ---- constants ----
            c_W = [cpool.tile([F, F], dt.float32, tag=f"W{i}", name=f"cW{i}") for i in range(3)]
            c_b = [cpool.tile([128, F], dt.float32, tag=f"b{i}", name=f"cb{i}") for i in range(3)]
            c_dinv = cpool.tile([128, N_WIN], dt.float32, tag="dinv", name="dinv")
            c_iota3 = cpool.tile([128, 128, NBMAX], dt.bfloat16, tag="iota3", name="iota3")
            c_ilo = cpool.tile([128, n_lo // 16], dt.int16, tag="ilo", name="ilo")
            c_ihi = cpool.tile([128, n_hi // 16], dt.int16, tag="ihi", name="ihi")
            c_dlo = cpool.tile([128, n_lo // 128], dt.bfloat16, tag="dlo", name="dlo")
            c_dhi = cpool.tile([128, n_hi // 128], dt.bfloat16, tag="dhi", name="dhi")
            c_ident = cpool.tile([128, 128], dt.float32, tag="ident", name="ident")
            c_W16 = [cpool.tile([F, F], dt.bfloat16, tag=f"W16_{i}", name=f"cW16_{i}")
                     for i in range(3)]
            c_ident16 = cpool.tile([128, 128], dt.bfloat16, tag="id16", name="id16")
            for i in range(3):
                nc.sync.dma_start(c_W[i][:], t_W[i][:])
                nc.sync.dma_start(c_b[i][:], t_b[i][:])
            nc.sync.dma_start(c_dinv[:], t_dinv[:])
            nc.sync.dma_start(c_iota3[:], t_iota3[:])
            nc.sync.dma_start(c_ilo[:], t_ilo[:])
            nc.sync.dma_start(c_ihi[:], t_ihi[:])
            nc.sync.dma_start(c_dlo[:], t_dlo[:])
            nc.sync.dma_start(c_dhi[:], t_dhi[:])
            make_identity(nc, c_ident[:])
            nc.vector.tensor_copy(out=c_ident16[:], in_=c_ident[:])
            for i in range(3):
                nc.vector.tensor_copy(out=c_W16[i][:], in_=c_W[i][:])

            # ---- persistent state ----
            hT = [spool.tile([128, PER_CORE], dt.bfloat16, tag="hT_a", name="hT_a"),
                  spool.tile([128, PER_CORE], dt.bfloat16, tag="hT_b", name="hT_b")]
            nc.sync.dma_start(hT[0][:], t_xT[:])
            y_sb = spool.tile([128, N_WIN, F], dt.bfloat16, tag="y_sb", name="y_sb")
            out_sb = spool.tile([128, N_WIN, F_OUT], dt.float32, tag="out_sb", name="out_sb")

            y_As = [dpool.tile([A_ROWS, F], dt.bfloat16, addr_space="Shared",
                               name=f"y_A{i}") for i in range(3)]
            y_Bs = [dpool.tile([B_ROWS, F], dt.bfloat16, addr_space="Shared",
                               name=f"y_B{i}") for i in range(3)]
            ag_As = [dpool.tile([A_PC, F], dt.bfloat16, name=f"ag_A{i}")
                     for i in range(3)]
            ag_Bs = [dpool.tile([B_PC, F], dt.bfloat16, name=f"ag_B{i}")
                     for i in range(3)]

            def emit_y(layer, w):
                """y[w] = dinv * (h @ W[layer]) for layer's aggregation, plus
                the per-window push into the exchange staging buffer and the
                half-table AllGather when a half completes."""
                h_in = hT[layer % 2]
                ps = psA.tile([128, F], dt.float32, tag="psA", space="PSUM")
                nc.tensor.matmul(ps[:], lhsT=h_in[:, w * 128:(w + 1) * 128],
                                 rhs=c_W16[layer][:], start=True, stop=True)
                nc.scalar.activation(
                    y_sb[:, w, :], ps[:], mybir.ActivationFunctionType.Copy,
                    scale=c_dinv[:, w:w + 1])
                if w < A_WINS:
                    agv = ag_As[layer][:].rearrange("(t p) f -> p t f", p=128)
                    nc.sync.dma_start(agv[:, w:w + 1, :], y_sb[:, w:w + 1, :])
                else:
                    agv = ag_Bs[layer][:].rearrange("(t p) f -> p t f", p=128)
                    nc.sync.dma_start(agv[:, w - A_WINS:w - A_WINS + 1, :],
                                      y_sb[:, w:w + 1, :])
                if w == A_WINS - 1:
                    nc.gpsimd.collective_compute(
                        "AllGather", mybir.AluOpType.bypass,
                        replica_groups=[list(range(N_CORES))],
                        ins=[ag_As[layer].opt()], outs=[y_As[layer].opt()])
                elif w == N_WIN - 1:
                    nc.gpsimd.collective_compute(
                        "AllGather", mybir.AluOpType.bypass,
                        replica_groups=[list(range(N_CORES))],
                        ins=[ag_Bs[layer].opt()], outs=[y_Bs[layer].opt()])

            def build_S_group(gi):
                """One-hot scatter blocks for a whole group, one is_equal per
                stream: S[e, d, j] = (dl[e, base+j] == d), bf16."""
                out = {}
                for st, (a, b, dl_t, cap) in enumerate((
                        (glo[gi][0], glo[gi][1], c_dlo, cap_lo),
                        (ghi[gi][0], ghi[gi][1], c_dhi, cap_hi))):
                    nb = (b - a) // 128
                    S = sbld.tile([128, 128, cap], dt.bfloat16,
                                  tag=f"S{st}", name=f"S{st}")
                    dl_b = dl_t[:, a // 128:b // 128].unsqueeze(1).broadcast_to(
                        [128, 128, nb])
                    nc.vector.tensor_tensor(
                        out=S[:, :, :nb], in0=dl_b, in1=c_iota3[:, :, :nb],
                        op=mybir.AluOpType.is_equal)
                    out[st] = S
                return out

            # ---- layer 0 phase A (prologue): per-window y, but batched
            # staging DMAs (one per half) so the Sync queue is not the
            # serial bottleneck before AG_A can fire ----
            for t in range(N_WIN):
                ps = psA.tile([128, F], dt.float32, tag="psA", space="PSUM")
                nc.tensor.matmul(ps[:], lhsT=hT[0][:, t * 128:(t + 1) * 128],
                                 rhs=c_W16[0][:], start=True, stop=True)
                nc.scalar.activation(
                    y_sb[:, t, :], ps[:], mybir.ActivationFunctionType.Copy,
                    scale=c_dinv[:, t:t + 1])
                if t == A_WINS - 1:
                    agv = ag_As[0][:].rearrange("(t p) f -> p t f", p=128)
                    nc.sync.dma_start(agv[:], y_sb[:, :A_WINS, :])
                    nc.gpsimd.collective_compute(
                        "AllGather", mybir.AluOpType.bypass,
                        replica_groups=[list(range(N_CORES))],
                        ins=[ag_As[0].opt()], outs=[y_As[0].opt()])
                elif t == N_WIN - 1:
                    agv = ag_Bs[0][:].rearrange("(t p) f -> p t f", p=128)
                    nc.sync.dma_start(agv[:], y_sb[:, A_WINS:, :])
                    nc.gpsimd.collective_compute(
                        "AllGather", mybir.AluOpType.bypass,
                        replica_groups=[list(range(N_CORES))],
                        ins=[ag_Bs[0].opt()], outs=[y_Bs[0].opt()])

            S_next = build_S_group(0)
            for layer in range(3):
                h_out = hT[(layer + 1) % 2]
                y_A, y_B = y_As[layer], y_Bs[layer]
                # ---- phase B: gather + one-hot matmul aggregation ----
                for gi, g in enumerate(groups):
                    lo_a, lo_b = glo[gi]
                    hi_a, hi_b = ghi[gi]
                    m_lo = gpool.tile([128, cap_lo, F], dt.bfloat16, tag="mlo", name="mlo")
                    m_hi = gpool.tile([128, cap_hi, F], dt.bfloat16, tag="mhi", name="mhi")
                    # balance all 4 queues to ~T/4 blocks each, splitting
                    # at block granularity across both streams
                    L = (lo_b - lo_a) // 128
                    H = (hi_b - hi_a) // 128
                    T = L + H
                    marks = sorted(set(
                        min(T, max(0, (T * k + 3) // 4)) for k in range(5)))
                    pieces = []  # (stream, blk_start, blk_end, queue)
                    for q in range(len(marks) - 1):
                        a_blk, b_blk = marks[q], marks[q + 1]
                        if b_blk <= a_blk:
                            continue
                        lo_s, lo_e = min(a_blk, L), min(b_blk, L)
                        if lo_e > lo_s:
                            pieces.append((0, lo_s, lo_e, q))
                        hi_s, hi_e = max(a_blk - L, 0), max(b_blk - L, 0)
                        if hi_e > hi_s:
                            pieces.append((1, hi_s, hi_e, q))
                    for st, b0, b1, q in pieces:
                        if st == 0:
                            aa, bb = lo_a + b0 * 128, lo_a + b1 * 128
                            nc.gpsimd.dma_gather(
                                out_ap=m_lo[:, b0:b1, :], in_ap=y_A[:],
                                idxs_ap=c_ilo[:, aa // 16:bb // 16],
                                num_idxs=bb - aa, num_idxs_reg=bb - aa,
                                elem_size=F, queue_num=q, single_packet=False)
                        else:
                            aa, bb = hi_a + b0 * 128, hi_a + b1 * 128
                            nc.gpsimd.dma_gather(
                                out_ap=m_hi[:, b0:b1, :], in_ap=y_B[:],
                                idxs_ap=c_ihi[:, aa // 16:bb // 16],
                                num_idxs=bb - aa, num_idxs_reg=bb - aa,
                                elem_size=F, queue_num=q, single_packet=False)
                    S_cur, S_next = S_next, None
                    nxt = gi + 1 if gi + 1 < len(groups) else (
                        0 if layer < 2 else None)
                    if nxt is not None:
                        S_next = build_S_group(nxt)
                    # pass 1: aggregation matmuls + h epilogue per window
                    hbs = {}
                    for w in g:
                        nb_lo = int(blk_lo[w])
                        nb_hi = int(blk_hi[w])
                        nblk = nb_lo + nb_hi
                        B_lo = int(off_lo[w]) // 128 - lo_a // 128
                        B_hi = int(off_hi[w]) // 128 - hi_a // 128
                        agg = psB.tile([128, F], dt.float32, tag="agg", space="PSUM")
                        nblk += 1
                        nc.tensor.matmul(agg[:], lhsT=c_ident16[:],
                                         rhs=y_sb[:, w, :], start=True,
                                         stop=False)
                        k = 1
                        for j in range(nb_lo):
                            nc.tensor.matmul(
                                agg[:], lhsT=S_cur[0][:, :, B_lo + j],
                                rhs=m_lo[:, B_lo + j, :],
                                start=(k == 0), stop=(k == nblk - 1))
                            k += 1
                        for j in range(nb_hi):
                            nc.tensor.matmul(
                                agg[:], lhsT=S_cur[1][:, :, B_hi + j],
                                rhs=m_hi[:, B_hi + j, :],
                                start=(k == 0), stop=(k == nblk - 1))
                            k += 1
                        # h = dinv*agg + b
                        hb = wpool.tile([128, F], dt.float32, tag="hb", name="hb")
                        nc.vector.scalar_tensor_tensor(
                            out=hb[:], in0=agg[:],
                            scalar=c_dinv[:, w:w + 1], in1=c_b[layer][:],
                            op0=mybir.AluOpType.mult, op1=mybir.AluOpType.add)
                        hbs[w] = hb
                    # pass 2: transpose h, stage y for the next layer
                    for w in g:
                        hb = hbs[w]
                        if layer < 2:
                            tp = psT.tile([128, 128], dt.float32, tag="tp", space="PSUM")
                            nc.tensor.transpose(tp[:], hb[:], c_ident[:])
                            nc.scalar.copy(
                                h_out[:, w * 128:(w + 1) * 128], tp[:])
                            # phase-A fold: y for layer+1 from the fresh h
                            emit_y(layer + 1, w)
                        else:
                            nc.scalar.activation(
                                out_sb[:, w, :], hb[:, :F_OUT],
                                mybir.ActivationFunctionType.Relu)
            nc.sync.dma_start(
                t_out[:].rearrange("(t p) f -> p t f", p=128), out_sb[:])

    nc.compile()

    # ---- per-core inputs ----
    xT_all = np.zeros((128, N_PAD), np.float32)
    xT_all[:, new_pos] = np.asarray(x, np.float32).T
    iota3 = np.broadcast_to(
        np.arange(128, dtype=np.float32)[None, :, None],
        (128, 128, NBMAX)).astype(BF16)
    in_maps = []
    for c in range(N_CORES):
        rows = slice(c * PER_CORE, (c + 1) * PER_CORE)
        din = dinv_pad[rows].reshape(N_WIN, 128).T.copy()  # [128, N_WIN]
        in_map = {
            "xT_own": np.ascontiguousarray(xT_all[:, rows]).astype(BF16),
            "dinv_own": din,
            "iota3": iota3.copy(),
            "idx_lo": _wrap_idx16(idx_lo[c]),
            "idx_hi": _wrap_idx16(idx_hi[c]),
            "dl_lo": dl_lo[c].reshape(-1, 128).T.astype(BF16).copy(),
            "dl_hi": dl_hi[c].reshape(-1, 128).T.astype(BF16).copy(),
        }
        for i in range(3):
            in_map[f"W{i+1}m"] = Ws[i].copy()
            in_map[f"b{i+1}m"] = np.broadcast_to(bs[i], (128, F)).copy()
        in_maps.append(in_map)

    if sim:
        from concourse.bass_interp import MultiCoreSim
        mcs = MultiCoreSim(nc, num_cores=N_CORES, trace=False,
                           require_finite=False, require_nnan=False)
        for ci, core in enumerate(mcs.cores.values()):
            for k, v in in_maps[ci].items():
                core.tensor(k)[:] = v
        mcs.simulate(check_with_hw=False)
        outs = [np.asarray(core.tensor("h_out"))
                for core in mcs.cores.values()]
        res = None
    else:
        res = bass_utils.run_bass_kernel_spmd(
            nc, in_maps, core_ids=list(range(N_CORES)), trace=trace)
        outs = [r["h_out"] for r in res.results]
    full = np.concatenate(outs, axis=0)[new_pos]
    return full, res


def kernel(**inputs) -> np.ndarray:
    edge_index = np.asarray(inputs["edge_index"])
    prep = _preprocess(edge_index)
    out, _ = _build_and_run(inputs, *prep)
    return out


# revision 31
# speedup vs baseline: 1.0210x; 1.0210x over previous
"""GCN 3-layer kernel for Trainium2, 8-core SPMD.

Math (per layer, PyG GCN convention with self-loops, factorized):
    deg[d]  = indegree(d) + 1;  dinv = deg^-1/2
    y       = dinv[:,None] * (h @ W)                    (per-node scale)
    agg[d]  = sum_{e: dst[e]=d} y[src[e]]  + y[d]       (self-loop as edge)
    h_next  = dinv[:,None] * agg + b                    (+ relu on last layer)

Distribution: destination-sharded across 8 cores (6272 = 49*128 node slots
per core, padded to 50176 total).  The per-layer y table is exchanged in
TWO AllGather halves split by window (windows 0-24 -> table A of 25600
rows, windows 25-48 -> table B of 24576 rows) so the first half of the
exchange overlaps the tail of the previous layer's aggregation: y for
layer L+1 is computed inside layer L's per-window epilogue (phase-A fold),
and AG_A fires as soon as window 24's y is out.

Aggregation: per 128-dst window, gather message rows with dma_gather
(4 SWDGE queues; stream A split over q0/q1, stream B over q2/q3; indices
sorted by source row for DRAM page locality) and scatter-add them with
one-hot matmuls on the PE (PSUM accumulation per window).  One-hot blocks
S[e, d, j] = (dl[e, j] == d) are built in batches on the DVE: one
tensor_tensor(is_equal) per (window, stream) with a stride-0 broadcast of
dl against a materialized iota, consumed by the PE as strided slices
S[:, :, j].

Gather indices are int16: both tables are < 32768 rows, so no base-offset
tricks are needed.
"""

import numpy as np
import ml_dtypes

N_NODES = 50000
N_CORES = 8
PER_CORE = 6272            # 49 * 128
N_PAD = PER_CORE * N_CORES # 50176
N_WIN = PER_CORE // 128    # 49
A_WINS = 32                # windows 0..31 -> table A (32*128*8 = 32768 rows)
B_WINS = N_WIN - A_WINS    # windows 32..48 -> table B
A_PC = A_WINS * 128        # 3200 rows per core in A
B_PC = B_WINS * 128        # 3072 rows per core in B
A_ROWS = A_PC * N_CORES    # 25600
B_ROWS = B_PC * N_CORES    # 24576
F = 128                    # feature width (layer3 padded 64->128)
F_OUT = 64
GROUP_WINDOWS = 3          # windows per gather chunk

BF16 = ml_dtypes.bfloat16


def _wrap_idx16(idx: np.ndarray) -> np.ndarray:
    """Wrap a flat int16 index stream into the [128, n/16] layout dma_gather
    expects (element i at [i%16, i//16], replicated across the 8 groups of
    16 partitions)."""
    n = len(idx)
    assert n % 128 == 0
    cols = n // 16
    out = np.empty((128, cols), np.int16)
    w = idx.reshape(cols, 16).T  # [16, cols]
    for g in range(8):
        out[g * 16:(g + 1) * 16, :] = w
    return out


def _preprocess(edge_index: np.ndarray):
    """Host-side graph prep: degree norm, dst-sharding, per-window edge
    streams (A/B by source window), block padding shared across cores.

    Nodes are dealt into (core, window) slots round-robin by in-degree so
    every window carries a near-equal edge load across cores -- the shared
    (max-over-core) block padding shrinks accordingly."""
    src0 = edge_index[0].astype(np.int64)
    dst0 = edge_index[1].astype(np.int64)
    deg0 = np.bincount(dst0, minlength=N_NODES)
    order_n = np.argsort(-deg0, kind="stable")
    nbuck = N_CORES * N_WIN
    i = np.arange(N_NODES)
    bw = i % nbuck
    rank = i // nbuck
    slots = (bw // N_WIN) * PER_CORE + (bw % N_WIN) * 128 + rank
    new_pos = np.empty(N_NODES, np.int64)
    new_pos[order_n] = slots
    src = new_pos[src0]
    dst = new_pos[dst0]
    deg = np.bincount(dst, minlength=N_PAD).astype(np.float64) + 1.0
    dinv_pad = (1.0 / np.sqrt(deg)).astype(np.float32)

    # self-loops are applied on-chip from the local y window (no gather)
    src_a, dst_a = src, dst

    core_of = dst_a // PER_CORE
    win_of = (dst_a % PER_CORE) // 128
    dloc_of = dst_a % 128
    src_core = src_a // PER_CORE
    src_loc = src_a % PER_CORE
    is_lo = src_loc < A_PC          # stream A: source window < 25
    # relabeled gather rows in each half-table
    gidx_a = src_core * A_PC + src_loc
    gidx_b = src_core * B_PC + (src_loc - A_PC)

    # sort once by (core, window)
    order = np.lexsort((dst_a, win_of, core_of))
    core_s, win_s, dloc_s, lo_s = (
        core_of[order], win_of[order], dloc_of[order], is_lo[order])
    ga_s, gb_s = gidx_a[order], gidx_b[order]

    # per (core, window, stream) counts
    counts = np.zeros((N_CORES, N_WIN, 2), np.int64)
    np.add.at(counts, (core_s, win_s, (~lo_s).astype(np.int64)), 1)
    # shared block counts per window (max over cores), at least 1 block
    blk_lo = np.maximum(1, -(-counts[:, :, 0].max(axis=0) // 128))  # [N_WIN]
    blk_hi = np.maximum(1, -(-counts[:, :, 1].max(axis=0) // 128))  # [N_WIN]

    # slot offsets within each stream
    off_lo = np.concatenate([[0], np.cumsum(blk_lo * 128)])
    off_hi = np.concatenate([[0], np.cumsum(blk_hi * 128)])
    n_lo, n_hi = int(off_lo[-1]), int(off_hi[-1])

    # fill per-core padded streams
    idx_lo = np.zeros((N_CORES, n_lo), np.int16)
    idx_hi = np.zeros((N_CORES, n_hi), np.int16)
    dl_lo = np.full((N_CORES, n_lo), 999.0, np.float32)
    dl_hi = np.full((N_CORES, n_hi), 999.0, np.float32)

    # boundaries of (core, window) groups in the sorted arrays
    keys = core_s * N_WIN + win_s
    bounds = np.searchsorted(keys, np.arange(N_CORES * N_WIN + 1))
    for c in range(N_CORES):
        for w in range(N_WIN):
            k = c * N_WIN + w
            sl = slice(bounds[k], bounds[k + 1])
            s_dl = dloc_s[sl]; s_lo = lo_s[sl]
            lo_src = ga_s[sl][s_lo]; lo_dl = s_dl[s_lo]
            hi_src = gb_s[sl][~s_lo]; hi_dl = s_dl[~s_lo]
            # sort each stream by source row: ascending DRAM addresses give
            # the memory controller page locality during the gather
            o_lo = np.argsort(lo_src, kind="stable")
            lo_src, lo_dl = lo_src[o_lo], lo_dl[o_lo]
            o_hi = np.argsort(hi_src, kind="stable")
            hi_src, hi_dl = hi_src[o_hi], hi_dl[o_hi]
            o = off_lo[w]
            idx_lo[c, o:o + len(lo_src)] = lo_src.astype(np.int16)
            dl_lo[c, o:o + len(lo_src)] = lo_dl
            o = off_hi[w]
            idx_hi[c, o:o + len(hi_src)] = hi_src.astype(np.int16)
            dl_hi[c, o:o + len(hi_src)] = hi_dl

    return (dinv_pad, blk_lo, blk_hi, off_lo, off_hi, idx_lo, idx_hi,
            dl_lo, dl_hi, new_pos)


def _build_and_run(inputs_np, dinv_pad, blk_lo, blk_hi, off_lo, off_hi,
                   idx_lo, idx_hi, dl_lo, dl_hi, new_pos,
                   trace=False, sim=False):
    import concourse.bacc as bacc
    import concourse.mybir as mybir
    from concourse.tile import TileContext
    from concourse import bass, bass_utils, library_config
    from concourse.masks import make_identity

    x = inputs_np["x"]
    Ws = [np.asarray(inputs_np[k], np.float32) for k in ("W1", "W2", "W3")]
    bs = [np.asarray(inputs_np[k], np.float32) for k in ("b1", "b2", "b3")]
    # pad W3/b3 to 128 output features
    W3p = np.zeros((F, F), np.float32); W3p[:, :F_OUT] = Ws[2]
    b3p = np.zeros(F, np.float32); b3p[:F_OUT] = bs[2]
    Ws[2], bs[2] = W3p, b3p

    n_lo, n_hi = idx_lo.shape[1], idx_hi.shape[1]
    # gather groups of GROUP_WINDOWS windows
    groups = [list(range(g, min(g + GROUP_WINDOWS, N_WIN)))
              for g in range(0, N_WIN, GROUP_WINDOWS)]
    glo = [(int(off_lo[g[0]]), int(off_lo[g[-1] + 1])) for g in groups]
    ghi = [(int(off_hi[g[0]]), int(off_hi[g[-1] + 1])) for g in groups]
    cap_lo = max(b - a for a, b in glo) // 128
    cap_hi = max(b - a for a, b in ghi) // 128
    NBMAX = max(cap_lo, cap_hi)   # iota depth for group-level S builds

    nc = bacc.Bacc("TRN2", target_bir_lowering=False, debug=False,
                   num_devices=N_CORES, num_swdge_queues=4,
                   dynamic_dma_scratch_size=24576)
    dt = mybir.dt

    # ---- kernel I/O -----------------------------------------------------
    t_xT = nc.dram_tensor("xT_own", [128, PER_CORE], dt.bfloat16, kind="ExternalInput")
    t_W = [nc.dram_tensor(f"W{i+1}m", [F, F], dt.float32, kind="ExternalInput") for i in range(3)]
    t_b = [nc.dram_tensor(f"b{i+1}m", [128, F], dt.float32, kind="ExternalInput") for i in range(3)]
    t_dinv = nc.dram_tensor("dinv_own", [128, N_WIN], dt.float32, kind="ExternalInput")
    t_iota3 = nc.dram_tensor("iota3", [128, 128, NBMAX], dt.bfloat16, kind="ExternalInput")
    t_ilo = nc.dram_tensor("idx_lo", [128, n_lo // 16], dt.int16, kind="ExternalInput")
    t_ihi = nc.dram_tensor("idx_hi", [128, n_hi // 16], dt.int16, kind="ExternalInput")
    t_dlo = nc.dram_tensor("dl_lo", [128, n_lo // 128], dt.bfloat16, kind="ExternalInput")
    t_dhi = nc.dram_tensor("dl_hi", [128, n_hi // 128], dt.bfloat16, kind="ExternalInput")
    t_out = nc.dram_tensor("h_out", [PER_CORE, F_OUT], dt.float32, kind="ExternalOutput")

    with TileContext(nc) as tc:
        nc.gpsimd.load_library(library_config.mlp)
        with tc.tile_pool(name="const", bufs=1) as cpool, \
             tc.tile_pool(name="state", bufs=1) as spool, \
             tc.tile_pool(name="gath", bufs=4) as gpool, \
             tc.tile_pool(name="sbld", bufs=2) as sbld, \
             tc.tile_pool(name="work", bufs=7) as wpool, \
             tc.tile_pool(name="psA", bufs=3, space="PSUM") as psA, \
             tc.tile_pool(name="psB", bufs=3, space="PSUM") as psB, \
             tc.tile_pool(name="psT", bufs=2, space="PSUM") as psT, \
             tc.tile_pool(name="dram", bufs=1, space="DRAM") as dpool:

            # ---- constants ----
            c_W = [cpool.tile([F, F], dt.float32, tag=f"W{i}", name=f"cW{i}") for i in range(3)]
            c_b = [cpool.tile([128, F], dt.float32, tag=f"b{i}", name=f"cb{i}") for i in range(3)]
            c_dinv = cpool.tile([128, N_WIN], dt.float32, tag="dinv", name="dinv")
            c_iota3 = cpool.tile([128, 128, NBMAX], dt.bfloat16, tag="iota3", name="iota3")
            c_ilo = cpool.tile([128, n_lo // 16], dt.int16, tag="ilo", name="ilo")
            c_ihi = cpool.tile([128, n_hi // 16], dt.int16, tag="ihi", name="ihi")
            c_dlo = cpool.tile([128, n_lo // 128], dt.bfloat16, tag="dlo", name="dlo")
            c_dhi = cpool.tile([128, n_hi // 128], dt.bfloat16, tag="dhi", name="dhi")
            c_ident = cpool.tile([128, 128], dt.float32, tag="ident", name="ident")
            c_W16 = [cpool.tile([F, F], dt.bfloat16, tag=f"W16_{i}", name=f"cW16_{i}")
                     for i in range(3)]
            c_ident16 = cpool.tile([128, 128], dt.bfloat16, tag="id16", name="id16")
            for i in range(3):
                nc.sync.dma_start(c_W[i][:], t_W[i][:])
                nc.sync.dma_start(c_b[i][:], t_b[i][:])
            nc.sync.dma_start(c_dinv[:], t_dinv[:])
            nc.sync.dma_start(c_iota3[:], t_iota3[:])
            nc.sync.dma_start(c_ilo[:], t_ilo[:])
            nc.sync.dma_start(c_ihi[:], t_ihi[:])
            nc.sync.dma_start(c_dlo[:], t_dlo[:])
            nc.sync.dma_start(c_dhi[:], t_dhi[:])
            make_identity(nc, c_ident[:])
            nc.vector.tensor_copy(out=c_ident16[:], in_=c_ident[:])
            for i in range(3):
                nc.vector.tensor_copy(out=c_W16[i][:], in_=c_W[i][:])

            # ---- persistent state ----
            hT = [spool.tile([128, PER_CORE], dt.bfloat16, tag="hT_a", name="hT_a"),
                  spool.tile([128, PER_CORE], dt.bfloat16, tag="hT_b", name="hT_b")]
            nc.sync.dma_start(hT[0][:], t_xT[:])
            y_sb = spool.tile([128, N_WIN, F], dt.bfloat16, tag="y_sb", name="y_sb")
            out_sb = spool.tile([128, N_WIN, F_OUT], dt.float32, tag="out_sb", name="out_sb")

            y_As = [dpool.tile([A_ROWS, F], dt.bfloat16, addr_space="Shared",
                               name=f"y_A{i}") for i in range(3)]
            y_Bs = [dpool.tile([B_ROWS, F], dt.bfloat16, addr_space="Shared",
                               name=f"y_B{i}") for i in range(3)]
            ag_As = [dpool.tile([A_PC, F], dt.bfloat16, name=f"ag_A{i}")
                     for i in range(3)]
            ag_Bs = [dpool.tile([B_PC, F], dt.bfloat16, name=f"ag_B{i}")
                     for i in range(3)]

            def emit_y(layer, w):
                """y[w] = dinv * (h @ W[layer]) for layer's aggregation, plus
                the per-window push into the exchange staging buffer and the
                half-table AllGather when a half completes."""
                h_in = hT[layer % 2]
                ps = psA.tile([128, F], dt.float32, tag="psA", space="PSUM")
                nc.tensor.matmul(ps[:], lhsT=h_in[:, w * 128:(w + 1) * 128],
                                 rhs=c_W16[layer][:], start=True, stop=True)
                nc.scalar.activation(
                    y_sb[:, w, :], ps[:], mybir.ActivationFunctionType.Copy,
                    scale=c_dinv[:, w:w + 1])
                if w < A_WINS:
                    agv = ag_As[layer][:].rearrange("(t p) f -> p t f", p=128)
                    nc.sync.dma_start(agv[:, w:w + 1, :], y_sb[:, w:w + 1, :])
                else:
                    agv = ag_Bs[layer][:].rearrange("(t p) f -> p t f", p=128)
                    nc.sync.dma_start(agv[:, w - A_WINS:w - A_WINS + 1, :],
                                      y_sb[:, w:w + 1, :])
                if w == A_WINS - 1:
                    nc.gpsimd.collective_compute(
                        "AllGather", mybir.AluOpType.bypass,
                        replica_groups=[list(range(N_CORES))],
                        ins=[ag_As[layer].opt()], outs=[y_As[layer].opt()])
                elif w == N_WIN - 1:
                    nc.gpsimd.collective_compute(
                        "AllGather", mybir.AluOpType.bypass,
                        replica_groups=[list(range(N_CORES))],
                        ins=[ag_Bs[layer].opt()], outs=[y_Bs[layer].opt()])

            def build_S_group(gi):
                """One-hot scatter blocks for a whole group, one is_equal per
                stream: S[e, d, j] = (dl[e, base+j] == d), bf16."""
                out = {}
                for st, (a, b, dl_t, cap) in enumerate((
                        (glo[gi][0], glo[gi][1], c_dlo, cap_lo),
                        (ghi[gi][0], ghi[gi][1], c_dhi, cap_hi))):
                    nb = (b - a) // 128
                    S = sbld.tile([128, 128, cap], dt.bfloat16,
                                  tag=f"S{st}", name=f"S{st}")
                    dl_b = dl_t[:, a // 128:b // 128].unsqueeze(1).broadcast_to(
                        [128, 128, nb])
                    nc.vector.tensor_tensor(
                        out=S[:, :, :nb], in0=dl_b, in1=c_iota3[:, :, :nb],
                        op=mybir.AluOpType.is_equal)
                    out[st] = S
                return out

            # ---- layer 0 phase A (prologue): per-window y, but batched
            # staging DMAs (one per half) so the Sync queue is not the
            # serial bottleneck before AG_A can fire ----
            for t in range(N_WIN):
                ps = psA.tile([128, F], dt.float32, tag="psA", space="PSUM")
                nc.tensor.matmul(ps[:], lhsT=hT[0][:, t * 128:(t + 1) * 128],
                                 rhs=c_W16[0][:], start=True, stop=True)
                nc.scalar.activation(
                    y_sb[:, t, :], ps[:], mybir.ActivationFunctionType.Copy,
                    scale=c_dinv[:, t:t + 1])
                if t == A_WINS - 1:
                    agv = ag_As[0][:].rearrange("(t p) f -> p t f", p=128)
                    nc.sync.dma_start(agv[:], y_sb[:, :A_WINS, :])
                    nc.gpsimd.collective_compute(
                        "AllGather", mybir.AluOpType.bypass,
                        replica_groups=[list(range(N_CORES))],
                        ins=[ag_As[0].opt()], outs=[y_As[0].opt()])
                elif t == N_WIN - 1:
                    agv = ag_Bs[0][:].rearrange("(t p) f -> p t f", p=128)
                    nc.sync.dma_start(agv[:], y_sb[:, A_WINS:, :])
                    nc.gpsimd.collective_compute(
                        "AllGather", mybir.AluOpType.bypass,
                        replica_groups=[list(range(N_CORES))],
                        ins=[ag_Bs[0].opt()], outs=[y_Bs[0].opt()])

            S_next = build_S_group(0)
            for layer in range(3):
                h_out = hT[(layer + 1) % 2]
                y_A, y_B = y_As[layer], y_Bs[layer]
                # ---- phase B: gather + one-hot matmul aggregation ----
                for gi, g in enumerate(groups):
                    lo_a, lo_b = glo[gi]
                    hi_a, hi_b = ghi[gi]
                    m_lo = gpool.tile([128, cap_lo, F], dt.bfloat16, tag="mlo", name="mlo")
                    m_hi = gpool.tile([128, cap_hi, F], dt.bfloat16, tag="mhi", name="mhi")
                    # balance all 4 queues to ~T/4 blocks each, splitting
                    # at block granularity across both streams
                    L = (lo_b - lo_a) // 128
                    H = (hi_b - hi_a) // 128
                    T = L + H
                    marks = sorted(set(
                        min(T, max(0, (T * k + 3) // 4)) for k in range(5)))
                    pieces = []  # (stream, blk_start, blk_end, queue)
                    for q in range(len(marks) - 1):
                        a_blk, b_blk = marks[q], marks[q + 1]
                        if b_blk <= a_blk:
                            continue
                        lo_s, lo_e = min(a_blk, L), min(b_blk, L)
                        if lo_e > lo_s:
                            pieces.append((0, lo_s, lo_e, q))
                        hi_s, hi_e = max(a_blk - L, 0), max(b_blk - L, 0)
                        if hi_e > hi_s:
                            pieces.append((1, hi_s, hi_e, q))
                    for st, b0, b1, q0_ in pieces:
                        q = (q0_ + gi) % 4
                        if st == 0:
                            aa, bb = lo_a + b0 * 128, lo_a + b1 * 128
                            nc.gpsimd.dma_gather(
                                out_ap=m_lo[:, b0:b1, :], in_ap=y_A[:],
                                idxs_ap=c_ilo[:, aa // 16:bb // 16],
                                num_idxs=bb - aa, num_idxs_reg=bb - aa,
                                elem_size=F, queue_num=q, single_packet=False)
                        else:
                            aa, bb = hi_a + b0 * 128, hi_a + b1 * 128
                            nc.gpsimd.dma_gather(
                                out_ap=m_hi[:, b0:b1, :], in_ap=y_B[:],
                                idxs_ap=c_ihi[:, aa // 16:bb // 16],
                                num_idxs=bb - aa, num_idxs_reg=bb - aa,
                                elem_size=F, queue_num=q, single_packet=False)
                    S_cur, S_next = S_next, None
                    nxt = gi + 1 if gi + 1 < len(groups) else (
                        0 if layer < 2 else None)
                    if nxt is not None:
                        S_next = build_S_group(nxt)
                    # pass 1: aggregation matmuls + h epilogue per window
                    hbs = {}
                    for w in g:
                        nb_lo = int(blk_lo[w])
                        nb_hi = int(blk_hi[w])
                        nblk = nb_lo + nb_hi
                        B_lo = int(off_lo[w]) // 128 - lo_a // 128
                        B_hi = int(off_hi[w]) // 128 - hi_a // 128
                        agg = psB.tile([128, F], dt.float32, tag="agg", space="PSUM")
                        nblk += 1
                        nc.tensor.matmul(agg[:], lhsT=c_ident16[:],
                                         rhs=y_sb[:, w, :], start=True,
                                         stop=False)
                        k = 1
                        for j in range(nb_lo):
                            nc.tensor.matmul(
                                agg[:], lhsT=S_cur[0][:, :, B_lo + j],
                                rhs=m_lo[:, B_lo + j, :],
                                start=(k == 0), stop=(k == nblk - 1))
                            k += 1
                        for j in range(nb_hi):
                            nc.tensor.matmul(
                                agg[:], lhsT=S_cur[1][:, :, B_hi + j],
                                rhs=m_hi[:, B_hi + j, :],
                                start=(k == 0), stop=(k == nblk - 1))
                            k += 1
                        # h = dinv*agg + b
                        hb = wpool.tile([128, F], dt.float32, tag="hb", name="hb")
                        nc.vector.scalar_tensor_tensor(
                            out=hb[:], in0=agg[:],
                            scalar=c_dinv[:, w:w + 1], in1=c_b[layer][:],
                            op0=mybir.AluOpType.mult, op1=mybir.AluOpType.add)
                        hbs[w] = hb
                    # pass 2: transpose h, stage y for the next layer
                    for w in g:
                        hb = hbs[w]
                        if layer < 2:
                            tp = psT.tile([128, 128], dt.float32, tag="tp", space="PSUM")
                            nc.tensor.transpose(tp[:], hb[:], c_ident[:])
                            nc.scalar.copy(
                                h_out[:, w * 128:(w + 1) * 128], tp[:])
                            # phase-A fold: y for layer+1 from the fresh h
                            emit_y(layer + 1, w)
                        else:
                            nc.scalar.activation(
                                out_sb[:, w, :], hb[:, :F_OUT],
                                mybir.ActivationFunctionType.Relu)
            nc.sync.dma_start(
                t_out[:].rearrange("(t p) f -> p t f", p=128), out_sb[:])

    nc.compile()

    # ---- per-core inputs ----
    xT_all = np.zeros((128, N_PAD), np.float32)
    xT_all[:, new_pos] = np.asarray(x, np.float32).T
    iota3 = np.broadcast_to(
        np.arange(128, dtype=np.float32)[None, :, None],
        (128, 128, NBMAX)).astype(BF16)
    in_maps = []
    for c in range(N_CORES):
        rows = slice(c * PER_CORE, (c + 1) * PER_CORE)
        din = dinv_pad[rows].reshape(N_WIN, 128).T.copy()  # [128, N_WIN]
        in_map = {
            "xT_own": np.ascontiguousarray(xT_all[:, rows]).astype(BF16),
            "dinv_own": din,
            "iota3": iota3.copy(),
            "idx_lo": _wrap_idx16(idx_lo[c]),
            "idx_hi": _wrap_idx16(idx_hi[c]),
            "dl_lo": dl_lo[c].reshape(-1, 128).T.astype(BF16).copy(),
            "dl_hi": dl_hi[c].reshape(-1, 128).T.astype(BF16).copy(),
        }
        for i in range(3):
            in_map[f"W{i+1}m"] = Ws[i].copy()
            in_map[f"b{i+1}m"] = np.broadcast_to(bs[i], (128, F)).copy()
        in_maps.append(in_map)

    if sim:
        from concourse.bass_interp import MultiCoreSim
        mcs = MultiCoreSim(nc, num_cores=N_CORES, trace=False,
                           require_finite=False, require_nnan=False)
        for ci, core in enumerate(mcs.cores.values()):
            for k, v in in_maps[ci].items():
                core.tensor(k)[:] = v
        mcs.simulate(check_with_hw=False)
        outs = [np.asarray(core.tensor("h_out"))
                for core in mcs.cores.values()]
        res = None
    else:
        res = bass_utils.run_bass_kernel_spmd(
            nc, in_maps, core_ids=list(range(N_CORES)), trace=trace)
        outs = [r["h_out"] for r in res.results]
    full = np.concatenate(outs, axis=0)[new_pos]
    return full, res


def kernel(**inputs) -> np.ndarray:
    edge_index = np.asarray(inputs["edge_index"])
    prep = _preprocess(edge_index)
    out, _ = _build_and_run(inputs, *prep)
    return out
